# revision 29
# baseline (speedup 1.0000x reference)
"""Cross-attention (GQA) Trainium2 Bass kernel — v2.

Problem: B=2, Tq=Tkv=2048, D_MODEL=1024, 16 query heads / 4 kv heads,
head_dim=64.  Sharded over 8 NeuronCores as batch(2) x kv-group(4); each
core computes 4 query heads + its single kv head and a partial output
projection (Wo row-split by head group); partials are summed on host.

On-chip dataflow keeps activations "transposed" (feature dim on SBUF
partitions) end-to-end so that scores, softmax and P@V need no on-chip
transposes of large tensors:

  A: qT[e,t] = WqT.T @ xqT,  kvT = WkvT.T @ xcT        (f16, N=512)
     v[tk,dv] via PE-transpose of vT tiles
  B: ST[tk,tq] = kT.T @ qT_h ; two heads packed in the PE array via
     row-groups (K=64 each, h_even rows 0-63, h_odd rows 64-127)
  C: P = exp(ST/8): h_even tiles on ScalarE (exact spline exp),
     2/3 of h_odd tiles on VectorE via the Schraudolph int16 bit-trick
     (exp(x) ~= bitcast_f16(int16(x*1477.32/8 + 15315.75)), ~3% max rel
     err on those tiles; validated end-to-end rel err ~9.5e-3)
  D: outT'[dv|den, tq] = [v|1].T @ P ; ones-columns emit the softmax
     denominators in the same pass.  h_odd heads use [1|v] so their
     output rows land at partitions 64..127.
  normalize: 1/den via int32 bit-trick seed + one Newton step on VectorE
     (DVE reciprocal is ~6.5us/tile; Ln on ScalarE forces activation-table
     ping-pong at ~2.7us/switch).  Max den err 2.6e-3, validated on HW.
  E: yT += WoT_pair.T @ outT_norm (K=128: two heads stacked); E matmuls
     of block b are drip-fed into block b+1's loop to keep PE dense.

Emission is software-pipelined per t:  S(t), exp(t), D(t-1)  so the
in-order PE queue always has D(t-1)+S(t+1) to chew on while the exp of
S(t) runs on ScalarE/VectorE.  This keeps PE occupancy high enough that
the HAM clock gate stays at 8/8 (2.4 GHz); the v1 kernel sat at 4/8 for
~85% of its span.
"""

import os
import sys

import numpy as np

for _p in ("/opt/trn_rl_repo",):
    if _p not in sys.path and os.path.isdir(_p):
        sys.path.insert(0, _p)

import concourse.bass as bass
import concourse.bacc as bacc
import concourse.mybir as mybir
from concourse.tile import TileContext, add_dep_helper

# ---------------------------------------------------------------- problem dims
B = 2
TQ = 2048
TKV = 2048
D_MODEL = 1024
N_HEADS = 16
N_KV_HEADS = 4
HEAD_DIM = 64
N_CORES = 8
GROUPS = N_KV_HEADS  # kv groups = 4
HEADS_PER_DEV = N_HEADS // GROUPS  # 4
DQ = HEADS_PER_DEV * HEAD_DIM  # 256
DKV = 2 * HEAD_DIM  # 128 (k rows + v rows stacked)
SCALE = 1.0 / float(np.sqrt(HEAD_DIM))

P = 128
FREE = 512  # matmul moving-operand chunk
BLK = 1024  # tq block width (exp instruction width)

F32 = mybir.dt.float32
F16 = mybir.dt.float16
I16 = mybir.dt.int16
I32 = mybir.dt.int32

# Schraudolph f16 exp: exp(s/8) ~= bitcast_f16(int16(s*SCHRAUD_A + SCHRAUD_B))
SCHRAUD_A = float(1024.0 / np.log(2.0)) * SCALE  # 184.66495
SCHRAUD_B = float(15.0 * 1024.0 - 44.25)
RECIP_K = float(0x7EF311C3)  # reciprocal bit-trick magic


def build_bass():
    nc = bacc.Bacc()

    xq = nc.declare_dram_parameter("xqT", [D_MODEL, TQ], F16, isOutput=False)
    xc = nc.declare_dram_parameter("xcT", [D_MODEL, TKV], F16, isOutput=False)
    wq = nc.declare_dram_parameter("wqT", [D_MODEL, DQ], F16, isOutput=False)
    wkv = nc.declare_dram_parameter("wkvT", [D_MODEL, DKV], F16, isOutput=False)
    wo = nc.declare_dram_parameter("woT", [DQ, D_MODEL], F16, isOutput=False)
    cid = nc.declare_dram_parameter("cid", [P, P + 64], F16, isOutput=False)
    yt = nc.declare_dram_parameter("yT", [D_MODEL, TQ], F32, isOutput=True)

    DT = D_MODEL // P  # 8 d-tiles
    ET = DQ // P  # 2 e-tiles (query head pairs)
    NCH = TQ // FREE  # 4 chunks of 512
    NTK = TKV // P  # 16 tk tiles
    NBLK = TQ // BLK  # 2 tq blocks
    JPB = BLK // FREE  # 2 free-chunks per block
    MT = D_MODEL // P  # 8 output m-tiles

    Exp = mybir.ActivationFunctionType.Exp
    Mult = mybir.AluOpType.mult
    Add = mybir.AluOpType.add

    with TileContext(nc) as tc:
        with (
            tc.tile_pool(name="consts", bufs=1) as consts,
            tc.tile_pool(name="xch", bufs=6) as xpool,
            tc.tile_pool(name="pt", bufs=6) as ptpool,
            tc.tile_pool(name="nrm", bufs=2) as nrmpool,
            tc.tile_pool(name="yout", bufs=3) as ypool,
            tc.tile_pool(name="psA", bufs=2, space="PSUM") as psA,
            tc.tile_pool(name="psB", bufs=2, space="PSUM") as psB,
        ):
            # ---------------- constants / persistent tiles
            ident = consts.tile([P, P + 64], F16, tag="ident")
            nc.sync.dma_start(ident, cid[:])

            # pre-warm the exp activation table (~2.7us load) during the
            # A-phase DMAs instead of stalling the first real softmax exp
            warmup = consts.tile([P, 4], F16, tag="warmup")
            nc.scalar.activation(
                warmup,
                ident[:, :4],
                mybir.ActivationFunctionType.Exp,
                bias=0.0,
                scale=1.0,
            )

            wq_sb = consts.tile([P, DT, DQ], F16, tag="wq")
            nc.sync.dma_start(wq_sb, wq.rearrange("(i p) e -> p i e", p=P))
            wkv_sb = consts.tile([P, DT, DKV], F16, tag="wkv")
            nc.sync.dma_start(wkv_sb, wkv.rearrange("(i p) e -> p i e", p=P))
            wo_sb = consts.tile([P, ET, D_MODEL], F16, tag="wo")
            nc.sync.dma_start(wo_sb, wo.rearrange("(i p) m -> p i m", p=P))

            qt = consts.tile([P, ET, TQ], F16, tag="qt")  # qT: heads 2/tile
            kv = consts.tile([P, TKV], F16, tag="kv")  # rows 0-63 kT, 64-127 vT
            k2 = consts.tile([P, TKV], F16, tag="k2")  # rows 64-127 = kT copy
            vp = consts.tile([P, NTK, P], F16, tag="vp")  # [v | ones]
            vp2 = consts.tile([P, NTK, P], F16, tag="vp2")  # [ones | v]
            outs = consts.tile([P, ET, TQ], F16, tag="outs")  # normalized outT

            # ones halves of vp/vp2: one memset each, v halves written later
            nc.vector.memset(vp, 1.0)
            nc.vector.memset(vp2, 1.0)

            # ---------------- stage A: projections (weights stationary)
            # kv first (every BCD iteration needs the full kT/vT), then q
            for c in range(NCH):
                cs = slice(c * FREE, (c + 1) * FREE)
                xc_t = xpool.tile([P, DT, FREE], F16, tag="xch")
                nc.sync.dma_start(
                    xc_t, xc.rearrange("(i p) t -> p i t", p=P)[:, :, cs]
                )
                pkv = psB.tile([P, FREE], F32, tag="psB")
                for i in range(DT):
                    nc.tensor.matmul(
                        pkv,
                        (wkv_sb[:, i, :]),
                        (xc_t[:, i, :]),
                        start=(i == 0),
                        stop=(i == DT - 1),
                    )
                nc.scalar.copy(kv[:, cs], pkv)
                # duplicate kT rows into partitions 64..127 for row-packing
                nc.sync.dma_start(k2[HEAD_DIM : 2 * HEAD_DIM, cs], kv[:HEAD_DIM, cs])

            def q_chunk_load(c):
                cs = slice(c * FREE, (c + 1) * FREE)
                xq_t = xpool.tile([P, DT, FREE], F16, tag="xch", name="xq_t")
                nc.sync.dma_start(
                    xq_t, xq.rearrange("(i p) t -> p i t", p=P)[:, :, cs]
                )
                return xq_t

            def q_chunk_proj(c, xq_t, e):
                cs = slice(c * FREE, (c + 1) * FREE)
                pq = psA.tile([P, FREE], F32, tag="psA", name="pq")
                for i in range(DT):
                    nc.tensor.matmul(
                        pq,
                        (wq_sb[:, i, e * P : (e + 1) * P]),
                        (xq_t[:, i, :]),
                        start=(i == 0),
                        stop=(i == DT - 1),
                    )
                nc.scalar.copy(qt[:, e, cs], pq)

            # chunks 0/1 (needed for block 0) projected up front; chunks
            # 2/3 DMA-started now but their matmuls dripped into block 0's
            # loop so the PE reaches the softmax pipeline sooner
            for c in range(2):
                xq_t = q_chunk_load(c)
                for e in range(ET):
                    q_chunk_proj(c, xq_t, e)
            qpieces = []
            for c in range(2, NCH):
                xq_t = q_chunk_load(c)
                for e in range(ET):
                    qpieces.append((c, xq_t, e))

            # v' tiles: PE-transpose vT[64, tk*128 ..] -> [128, 64]; ones
            # halves were memset above.
            for t in range(NTK):
                ts_ = slice(t * P, (t + 1) * P)
                pv = psB.tile([P, HEAD_DIM], F16, tag="psB", name="pv")
                nc.tensor.transpose(
                    pv, kv[HEAD_DIM : 2 * HEAD_DIM, ts_], ident[HEAD_DIM:, HEAD_DIM:P]
                )
                nc.scalar.copy(vp[:, t, :HEAD_DIM], pv)
                nc.scalar.copy(vp2[:, t, HEAD_DIM:], pv)

            # -------- stage E piece emitter (drip-fed into the BCD stream)
            # width-1024 pieces (two tq chunks per PSUM-slot pass) halve the
            # number of PSUM-slot rotations and PSUM->SBUF copies vs 512
            def emit_out_piece(c, m, nch=1):
                cs = slice(c * FREE, (c + nch) * FREE)
                ms = slice(m * P, (m + 1) * P)
                py = psA.tile([P, nch * FREE], F32, tag="psA", name="py")
                for ci in range(nch):
                    co = slice(ci * FREE, (ci + 1) * FREE)
                    ccs = slice((c + ci) * FREE, (c + ci + 1) * FREE)
                    for ee in range(ET):
                        nc.tensor.matmul(
                            py[:, co],
                            (wo_sb[:, ee, ms]),
                            (outs[:, ee, ccs]),
                            start=(ee == 0),
                            stop=(ee == ET - 1),
                        )
                yo = ypool.tile([P, nch * FREE], F32, tag="yout", name="yo")
                nc.any.tensor_copy(yo, py)
                nc.sync.dma_start(yt[ms, cs], yo)

            # ---------------- stages B/C/D: attention per head-pair
            # Software-pipelined emission per t: S(t), exp(t), D(t-1); the
            # in-order PE queue then always has D(t-1) + S(t+1) available
            # while exp(t) runs on ACT/DVE.
            epieces = []  # deferred E pieces, drip-fed into the next block
            last_exps = [None, None]  # previous period's exp instructions
            last_dve_box = [False]  # was the previous h1 exp on DVE?

            for blk in range(NBLK):
                for e in range(ET):  # head pair (h_even=2e, h_odd=2e+1)
                    bs = slice(blk * BLK, (blk + 1) * BLK)
                    pd = [
                        psB.tile([P, BLK], F32, tag="psB", name=f"pd{_h}")
                        for _h in range(2)
                    ]  # D accumulators: [0]=h_even rows 0-63, [1]=h_odd

                    pts = [None] * NTK  # per-t pt tile pair

                    def emit_S(t):
                        ts_ = slice(t * P, (t + 1) * P)
                        pb = [
                            psA.tile([P, BLK], F32, tag="psA", name=f"pb{_h}")
                            for _h in range(2)
                        ]
                        for j in range(JPB):
                            js = slice(
                                blk * BLK + j * FREE, blk * BLK + (j + 1) * FREE
                            )
                            jo = slice(j * FREE, (j + 1) * FREE)
                            # scores, 2 heads row-packed (K=64 each, rows
                            # 0-63 / 64-127 -> concurrent on the PE).
                            # Cross-tie each matmul to the OTHER head's
                            # previous exp (its natural dep is only its own
                            # slot's exp): both become ready at the same
                            # tick, stay adjacent in the PE queue, and
                            # co-execute in disjoint row groups.  Without
                            # this the scheduler splits the pair and every
                            # score matmul runs solo (measured 2.14us/t).
                            m0 = nc.tensor.matmul(
                                pb[0][:, jo],
                                (kv[:HEAD_DIM, ts_]),
                                (qt[:HEAD_DIM, e, js]),
                            )
                            m1 = nc.tensor.matmul(
                                pb[1][:, jo],
                                (k2[HEAD_DIM:, ts_]),
                                (qt[HEAD_DIM:, e, js]),
                            )
                            # cross-tie for co-execution, but only when the
                            # previous h1 exp ran on DVE (in ACT-both
                            # periods the tie would chain h0 behind the
                            # serial second ACT exp — worse than unpaired)
                            if j == 0 and last_exps[0] is not None and last_dve_box[0]:
                                add_dep_helper(
                                    m0.ins, last_exps[1].ins, reason="tie h0<-e1"
                                )
                                add_dep_helper(
                                    m1.ins, last_exps[0].ins, reason="tie h1<-e0"
                                )
                        return pb

                    def emit_exp(t, pb):
                        pt = [
                            ptpool.tile([P, BLK], F16, tag="pt", name=f"pt{_h}")
                            for _h in range(2)
                        ]
                        # h_even: exact exp on ScalarE
                        i0 = nc.scalar.activation(
                            pt[0], pb[0], Exp, bias=0.0, scale=SCALE
                        )
                        # h_odd: 3/4 of tiles via DVE int16 bit-trick, split
                        # per j-half so pb1's slot frees after the j0 half
                        # and S(t+1)-j0 can start ~0.6us earlier
                        if t not in (0, 5, 10, 15):
                            i1 = None
                            for j in range(JPB):
                                jo = slice(j * FREE, (j + 1) * FREE)
                                ij = nc.vector.tensor_scalar(
                                    pt[1].bitcast(I16)[:, jo],
                                    pb[1][:, jo],
                                    SCHRAUD_A,
                                    SCHRAUD_B,
                                    mybir.AluOpType.mult,
                                    mybir.AluOpType.add,
                                )
                                if i1 is None:
                                    i1 = ij  # j0 half gates the next S pair
                            dve = True
                        else:
                            i1 = nc.scalar.activation(
                                pt[1], pb[1], Exp, bias=0.0, scale=SCALE
                            )
                            dve = False
                        pts[t] = pt
                        last_exps[0] = i0
                        last_exps[1] = i1
                        return dve

                    def emit_D(t):
                        pt = pts[t]
                        for h in range(2):
                            vo = vp if h == 0 else vp2
                            for j in range(JPB):
                                jo = slice(j * FREE, (j + 1) * FREE)
                                nc.tensor.matmul(
                                    pd[h][:, jo],
                                    vo[:, t, :],
                                    pt[h][:, jo],
                                    start=(t == 0),
                                    stop=(t == NTK - 1),
                                    skip_group_check=True,
                                )
                        pts[t] = None

                    for t in range(NTK):
                        pb = emit_S(t)
                        last_dve_box[0] = emit_exp(t, pb)
                        if t > 0:
                            emit_D(t - 1)
                        # drip stage-A q chunks 2/3 into block 0's loop
                        if qpieces and t in (2, 4, 6, 8):
                            q_chunk_proj(*qpieces.pop(0))
                        # drip merged E pieces of the previous block, one
                        # per ~3 periods; hold off early in e0 so the
                        # previous block's normalize chain has cleared
                        if epieces and t in (4, 7, 10, 13):
                            emit_out_piece(*epieces.pop(0))
                    emit_D(NTK - 1)

                    # ---- normalize: spill out/den rows into combined
                    # tiles, 1/den via bit-trick seed + 1 Newton (DVE only)
                    last = blk == NBLK - 1 and e == ET - 1
                    for jlo, jhi in ([(0, BLK)] if not last else [(0, FREE), (FREE, BLK)]):
                        jw = jhi - jlo
                        jsl = slice(jlo, jhi)
                        osl = slice(blk * BLK + jlo, blk * BLK + jhi)
                        rawc = nrmpool.tile([P, jw], F32, tag="rawc")
                        denc = nrmpool.tile([P, jw], F32, tag="denc")
                        # raw spills on DVE, den spills on ACT: the two pd
                        # banks release after ~2 parallel copies instead of
                        # 4 serial DVE ones (shrinks the e-boundary stall)
                        nc.vector.tensor_copy(rawc[:HEAD_DIM, :], pd[0][:HEAD_DIM, jsl])
                        nc.scalar.copy(denc[HEAD_DIM:, :], pd[0][HEAD_DIM:, jsl])
                        nc.vector.tensor_copy(rawc[HEAD_DIM:, :], pd[1][HEAD_DIM:, jsl])
                        nc.scalar.copy(denc[:HEAD_DIM, :], pd[1][:HEAD_DIM, jsl])
                        r0 = nrmpool.tile([P, jw], I32, tag="r0")
                        nc.vector.tensor_scalar(
                            r0, denc.bitcast(I32), -1.0, RECIP_K, Mult, Add
                        )
                        m = nrmpool.tile([P, jw], F32, tag="m")
                        nc.vector.tensor_tensor(m, denc, r0.bitcast(F32), Mult)
                        m2 = nrmpool.tile([P, jw], F32, tag="m2")
                        nc.vector.tensor_scalar(m2, m, -1.0, 2.0, Mult, Add)
                        rec = nrmpool.tile([P, jw], F32, tag="rec")
                        nc.vector.tensor_tensor(rec, r0.bitcast(F32), m2, Mult)
                        # swap halves so 1/den_h sits on its out_h partitions
                        rem = nrmpool.tile([P, jw], F32, tag="rem")
                        nc.sync.dma_start(rem[:HEAD_DIM, :], rec[HEAD_DIM:, :])
                        nc.sync.dma_start(rem[HEAD_DIM:, :], rec[:HEAD_DIM, :])
                        nc.vector.tensor_mul(outs[:, e, osl], rawc, rem)
                        if last:
                            # tail E pieces gated per tq half so they start
                            # as soon as their half of outs is normalized
                            c = blk * JPB + jlo // FREE
                            for mi in range(MT):
                                emit_out_piece(c, mi, 1)
                    if e == ET - 1 and not last:
                        for mi in range(MT):
                            epieces.append((blk * JPB, mi, JPB))

    nc.finalize()  # Bacc: runs wait-splitting/reg-alloc passes
    return nc


_NC_CACHE = None


def _get_nc():
    global _NC_CACHE
    if _NC_CACHE is None:
        _NC_CACHE = build_bass()
    return _NC_CACHE


def _cid():
    c = np.zeros((P, P + 64), dtype=np.float16)
    c[:, :P] = np.eye(P, dtype=np.float32)
    c[:, P:] = 1.0
    return c


def shard_inputs(query, context, Wq, Wk, Wv, Wo):
    """host-side sharding: 8 cores = batch(2) x kv-group(4)"""
    in_maps = []
    xqT = [np.ascontiguousarray(query[b].T).astype(np.float16) for b in range(B)]
    xcT = [np.ascontiguousarray(context[b].T).astype(np.float16) for b in range(B)]
    for core in range(N_CORES):
        b, g = divmod(core, GROUPS)
        wqT = np.ascontiguousarray(Wq[g * DQ : (g + 1) * DQ, :].T).astype(np.float16)
        wkvT = np.ascontiguousarray(
            np.concatenate(
                [
                    Wk[g * HEAD_DIM : (g + 1) * HEAD_DIM, :],
                    Wv[g * HEAD_DIM : (g + 1) * HEAD_DIM, :],
                ],
                axis=0,
            ).T
        ).astype(np.float16)
        woT = np.ascontiguousarray(Wo[:, g * DQ : (g + 1) * DQ].T).astype(np.float16)
        in_maps.append(
            {
                "xqT": xqT[b],
                "xcT": xcT[b],
                "wqT": wqT,
                "wkvT": wkvT,
                "woT": woT,
                "cid": _cid(),
            }
        )
    return in_maps


def kernel(query, context, Wq, Wk, Wv, Wo, _want_profile=False):
    from concourse.bass_utils import run_bass_kernel_spmd

    nc = _get_nc()
    in_maps = shard_inputs(query, context, Wq, Wk, Wv, Wo)
    res = run_bass_kernel_spmd(
        nc, in_maps, core_ids=list(range(N_CORES)), trace=_want_profile
    )
    out = np.zeros((B, TQ, D_MODEL), dtype=np.float32)
    for core in range(N_CORES):
        b = core // GROUPS
        out[b] += res.results[core]["yT"].T
    if _want_profile:
        return out, res
    return out
